# revision 27
# baseline (speedup 1.0000x reference)
"""Trainium2 Bass kernel for nn_BCNLayer (locally-connected 7x7 lattice layer + sigmoid).

Math: y[i,j,b] = sigmoid( sum_{dy,dx in [-3,3]} w[dy+3,dx+3][(i-dy)*W + (j-dx)]
                          * x[(i-dy)*W + (j-dx), b] )   (zero outside lattice)

Strategy:
  - 8-way shard over lattice rows (H=128 -> 16 dest rows/core, 22 source rows
    with 3-row halos, zero-padded at the edges).
  - For one dest row i and source-row offset d (7 of them), the contribution is
    a banded 128x128 matrix (band +-3 over lattice columns) applied to the
    source row's [128 cols x B batch] slab:  out[jd, b] += sum_js
    Wband[js, jd] * x[js, b].  That is exactly nc.tensor.matmul(psum, lhsT=Wband,
    rhs=xrow) accumulated over the 7 source rows.
  - Banded matrices are prebuilt on the host (numpy) and DMA'd in; HW executes
    pure DMA + matmul + sigmoid.
  - DMA plan: weights go on the Activation-engine HWDGE ring (fast spin-up, so
    the first LDWEIGHTS fires ~6us earlier than on the SWDGE), x on the SP ring,
    outputs grouped 4 dest-rows per DMA and split across both HWDGE rings.
    Output is bf16 (halves store traffic; sigmoid in bf16 keeps rel err ~2e-3).
"""

import os

import numpy as np

H = 128
W = 128
HW = H * W
B = 1024
NCORES = 8
T = H // NCORES  # dest rows per core = 16
S = T + 6        # source rows per core (halo 3 each side) = 22
BC = 512         # batch chunk (psum bank = 512 fp32)
NB = B // BC     # chunks = 2
OG = 4           # dest rows per output DMA

MM_MODE = os.environ.get("KERNEL_MM_MODE", "f16")

_cache: dict = {}

# filled by the last kernel() call when KERNEL_TRACE=1
last_exec_time_ns = None
last_results = None


def _build_program(mode: str):
    from contextlib import ExitStack

    import concourse.bacc as bacc
    import concourse.mybir as mybir
    import concourse.tile as tile

    nc = bacc.Bacc(
        "TRN2", target_bir_lowering=False, debug=False, num_devices=NCORES
    )
    mm_dt = {
        "f32": mybir.dt.float32,
        "f32r": mybir.dt.float32r,
        "f16": mybir.dt.float16,
    }[mode]
    # xs: [col, padded src row, batch]; wb: [t, src col, d*128 + dest col]
    xs = nc.dram_tensor("xs", [128, S, B], mm_dt, kind="ExternalInput").ap()
    wb = nc.dram_tensor(
        "wb", [T, 128, 7 * 128], mm_dt, kind="ExternalInput"
    ).ap()
    y = nc.dram_tensor(
        "y", [T, 128, B], mybir.dt.bfloat16, kind="ExternalOutput"
    ).ap()

    with tile.TileContext(nc) as tc, ExitStack() as ctx:
        xpool = ctx.enter_context(tc.tile_pool(name="x", bufs=1))
        wpool = ctx.enter_context(tc.tile_pool(name="w", bufs=1))
        mpool = ctx.enter_context(tc.tile_pool(name="m", bufs=1))
        ppool = ctx.enter_context(tc.tile_pool(name="ps", bufs=4, space="PSUM"))
        opool = ctx.enter_context(tc.tile_pool(name="o", bufs=1))

        xt = xpool.tile([128, S * B], mm_dt, tag="xslab")
        wt = wpool.tile([128, T * 7 * 128], mm_dt, tag="wslab")
        ob = opool.tile([128, NB * T * BC], mybir.dt.bfloat16, tag="o")
        xt3 = xt[:].rearrange("p (s b) -> p s b", s=S)

        # Warm the sigmoid ACT table now (it otherwise loads lazily right
        # before the first real sigmoid, stalling the pipeline).
        warm = mpool.tile([128, 1], mybir.dt.float32, tag="warm")
        nc.gpsimd.memset(warm[:], 0.0)
        nc.scalar.activation(warm[:], warm[:], mybir.ActivationFunctionType.Sigmoid)

        def _wb(eng, t0, t1):
            eng.dma_start(
                out=wt[:, t0 * 7 * 128 : t1 * 7 * 128].rearrange(
                    "p (t f) -> p t f", t=t1 - t0
                ),
                in_=wb[t0:t1].rearrange("t p f -> p t f"),
            )

        # The matmul-gating pieces ride the SP ring (steadier wake latency
        # than the Act ring), interleaved by need time with the x row pieces:
        # dest row t consumes wb[t] and padded x rows t..t+6 of both chunks
        # at ~t*3.2us into the stream. Late wb pieces and the outputs go on
        # the Act ring, where its wake/bandwidth variance is harmless.
        _wb(nc.sync, 0, 1)
        nc.sync.dma_start(out=xt3[:, 0:2, :], in_=xs[:, 0:2, :])
        _wb(nc.sync, 1, 3)
        for lo, hi in [(2, 4), (4, 6), (6, 8), (8, 10), (10, 13),
                       (13, 16), (16, 19), (19, 22)]:
            nc.sync.dma_start(
                out=xt3[:, lo:hi, :],
                in_=xs[:, lo:hi, :],
            )
        for t0, t1 in [(3, 6), (6, 9), (9, 12), (12, 16)]:
            _wb(nc.scalar, t0, t1)

        # t-outer / batch-chunk-inner: one stationary feeds both 512-col
        # chunks back-to-back (better LDWEIGHTS hiding), and output DMAs
        # cover the full batch. Last groups small for a quick final flush.
        ogroups = [(0, 4), (4, 8), (8, 12), (12, 14), (14, 15), (15, 16)]
        gidx = 0
        for t in range(T):
            ps = ppool.tile([128, NB * BC], mybir.dt.float32, tag="ps")
            for d in range(7):
                lhs = wt[:, (t * 7 + d) * 128 : (t * 7 + d + 1) * 128]
                for c in range(NB):
                    rhs = xt[
                        :, (t + d) * B + c * BC : (t + d) * B + (c + 1) * BC
                    ]
                    nc.tensor.matmul(
                        ps[:, c * BC : (c + 1) * BC],
                        lhs,
                        rhs,
                        start=(d == 0),
                        stop=(d == 6),
                    )
            nc.scalar.activation(
                ob[:, t * B : (t + 1) * B],
                ps[:],
                mybir.ActivationFunctionType.Sigmoid,
            )
            g0, g1 = ogroups[gidx]
            if t == g1 - 1:
                gidx += 1
                nc.scalar.dma_start(
                    out=y[g0:g1, :, :].rearrange("t p b -> p t b"),
                    in_=ob[:, g0 * NB * BC : g1 * NB * BC].rearrange(
                        "p (t b) -> p t b", t=g1 - g0
                    ),
                )
    nc.compile()
    return nc


def _build_banded(weights: np.ndarray) -> np.ndarray:
    """G[i, d, js, jd] = weight of edge (src row i+d-3, col js) -> (dest row i, col jd).

    dy = 3 - d (dest = src + dy), dx = jd - js, weight index = w[dy+3, dx+3][src_hw].
    """
    w4 = weights.reshape(7, 7, H, W)
    G = np.zeros((H, 7, W, W), np.float32)
    i = np.arange(H)
    for d in range(7):
        r = i + d - 3
        vi = i[(r >= 0) & (r < H)]
        if len(vi) == 0:
            continue
        for dxi in range(7):
            dx = dxi - 3
            js = np.arange(max(0, -dx), W - max(0, dx))
            G[vi[:, None], d, js[None, :], js[None, :] + dx] = w4[6 - d, dxi][
                (vi + d - 3)[:, None], js[None, :]
            ]
    return G


def kernel(x: np.ndarray, weights: np.ndarray) -> np.ndarray:
    global last_exec_time_ns, last_results
    from concourse.bass_utils import run_bass_kernel_spmd

    x = np.ascontiguousarray(x, dtype=np.float32)
    weights = np.ascontiguousarray(weights, dtype=np.float32)

    if MM_MODE not in _cache:
        _cache[MM_MODE] = _build_program(MM_MODE)
    nc = _cache[MM_MODE]

    io_dt = np.float16 if MM_MODE == "f16" else np.float32
    x3 = x.reshape(H, W, B)
    xp = np.zeros((H + 6, W, B), io_dt)
    xp[3 : H + 3] = x3.astype(io_dt)
    G = _build_banded(weights).astype(io_dt)  # [H, 7, W, W] = [i, d, js, jd]

    in_maps = []
    for q in range(NCORES):
        in_maps.append(
            {
                # [col, padded row, batch]
                "xs": np.ascontiguousarray(
                    xp[T * q : T * q + S].transpose(1, 0, 2)
                ),
                # [t, js, d*128 + jd]
                "wb": np.ascontiguousarray(
                    G[T * q : T * q + T].transpose(0, 2, 1, 3).reshape(
                        T, W, 7 * W
                    )
                ),
            }
        )

    trace = os.environ.get("KERNEL_TRACE", "0") == "1"
    res = run_bass_kernel_spmd(
        nc, in_maps, core_ids=list(range(NCORES)), trace=trace
    )
    last_exec_time_ns = res.exec_time_ns
    last_results = res
    out = np.concatenate(
        [
            np.asarray(r["y"]).astype(np.float32).reshape(T * W, B)
            for r in res.results
        ],
        axis=0,
    )
    return out


# revision 29
# speedup vs baseline: 1.0556x; 1.0556x over previous
"""Trainium2 Bass kernel for nn_BCNLayer (locally-connected 7x7 lattice layer + sigmoid).

Math: y[i,j,b] = sigmoid( sum_{dy,dx in [-3,3]} w[dy+3,dx+3][(i-dy)*W + (j-dx)]
                          * x[(i-dy)*W + (j-dx), b] )   (zero outside lattice)

Strategy:
  - 8-way shard over lattice rows (H=128 -> 16 dest rows/core, 22 source rows
    with 3-row halos, zero-padded at the edges).
  - For one dest row i and source-row offset d (7 of them), the contribution is
    a banded 128x128 matrix (band +-3 over lattice columns) applied to the
    source row's [128 cols x B batch] slab:  out[jd, b] += sum_js
    Wband[js, jd] * x[js, b].  That is exactly nc.tensor.matmul(psum, lhsT=Wband,
    rhs=xrow) accumulated over the 7 source rows.
  - Banded matrices are prebuilt on the host (numpy) and DMA'd in; HW executes
    pure DMA + matmul + sigmoid.
  - DMA plan: weights go on the Activation-engine HWDGE ring (fast spin-up, so
    the first LDWEIGHTS fires ~6us earlier than on the SWDGE), x on the SP ring,
    outputs grouped 4 dest-rows per DMA and split across both HWDGE rings.
    Output is bf16 (halves store traffic; sigmoid in bf16 keeps rel err ~2e-3).
"""

import os

import numpy as np

H = 128
W = 128
HW = H * W
B = 1024
NCORES = 8
T = H // NCORES  # dest rows per core = 16
S = T + 6        # source rows per core (halo 3 each side) = 22
BC = 512         # batch chunk (psum bank = 512 fp32)
NB = B // BC     # chunks = 2
OG = 4           # dest rows per output DMA

MM_MODE = os.environ.get("KERNEL_MM_MODE", "f16")

_cache: dict = {}

# filled by the last kernel() call when KERNEL_TRACE=1
last_exec_time_ns = None
last_results = None


def _build_program(mode: str):
    from contextlib import ExitStack

    import concourse.bacc as bacc
    import concourse.mybir as mybir
    import concourse.tile as tile

    nc = bacc.Bacc(
        "TRN2", target_bir_lowering=False, debug=False, num_devices=NCORES
    )
    mm_dt = {
        "f32": mybir.dt.float32,
        "f32r": mybir.dt.float32r,
        "f16": mybir.dt.float16,
    }[mode]
    # xs: [col, padded src row, batch]; wb: [t, src col, d*128 + dest col]
    xs = nc.dram_tensor("xs", [128, S, B], mm_dt, kind="ExternalInput").ap()
    wb = nc.dram_tensor(
        "wb", [T, 128, 7 * 128], mm_dt, kind="ExternalInput"
    ).ap()
    y = nc.dram_tensor(
        "y", [T, 128, B], mybir.dt.bfloat16, kind="ExternalOutput"
    ).ap()

    with tile.TileContext(nc) as tc, ExitStack() as ctx:
        xpool = ctx.enter_context(tc.tile_pool(name="x", bufs=1))
        wpool = ctx.enter_context(tc.tile_pool(name="w", bufs=1))
        mpool = ctx.enter_context(tc.tile_pool(name="m", bufs=1))
        ppool = ctx.enter_context(tc.tile_pool(name="ps", bufs=4, space="PSUM"))
        opool = ctx.enter_context(tc.tile_pool(name="o", bufs=1))

        xt = xpool.tile([128, S * B], mm_dt, tag="xslab")
        wt = wpool.tile([128, T * 7 * 128], mm_dt, tag="wslab")
        ob = opool.tile([128, NB * T * BC], mybir.dt.bfloat16, tag="o")
        xt3 = xt[:].rearrange("p (s b) -> p s b", s=S)

        def _wb(eng, t0, t1):
            eng.dma_start(
                out=wt[:, t0 * 7 * 128 : t1 * 7 * 128].rearrange(
                    "p (t f) -> p t f", t=t1 - t0
                ),
                in_=wb[t0:t1].rearrange("t p f -> p t f"),
            )

        # wb rides the Act ring in need-ordered pieces (wb[t] gates dest-row
        # t's LDWEIGHTS at ~t*3.2us into the stream); x rides the SP ring as
        # full-batch row-range pieces in need order (dest row t consumes
        # padded rows t..t+6 of both chunks). The ramp is supply-bound.
        _wb(nc.scalar, 0, 1)

        # Warm the sigmoid ACT table now — after the wb[0] post (the table
        # load inserts before the first ACTIVATE and must not delay wb[0]),
        # before the rest (it otherwise loads right before the first real
        # sigmoid, stalling the pipeline).
        warm = mpool.tile([128, 1], mybir.dt.float32, tag="warm")
        nc.gpsimd.memset(warm[:], 0.0)
        nc.scalar.activation(warm[:], warm[:], mybir.ActivationFunctionType.Sigmoid)

        for t0, t1 in [(1, 3), (3, 6), (6, 9), (9, 12), (12, 16)]:
            _wb(nc.scalar, t0, t1)
        for lo, hi in [(0, 2), (2, 4), (4, 6), (6, 8), (8, 10), (10, 13),
                       (13, 16), (16, 19), (19, 22)]:
            nc.sync.dma_start(
                out=xt3[:, lo:hi, :],
                in_=xs[:, lo:hi, :],
            )

        # t-outer / batch-chunk-inner: one stationary feeds both 512-col
        # chunks back-to-back (better LDWEIGHTS hiding), and output DMAs
        # cover the full batch. Last groups small for a quick final flush.
        ogroups = [(0, 4), (4, 8), (8, 12), (12, 14), (14, 15), (15, 16)]
        gidx = 0
        for t in range(T):
            ps = ppool.tile([128, NB * BC], mybir.dt.float32, tag="ps")
            for d in range(7):
                lhs = wt[:, (t * 7 + d) * 128 : (t * 7 + d + 1) * 128]
                for c in range(NB):
                    rhs = xt[
                        :, (t + d) * B + c * BC : (t + d) * B + (c + 1) * BC
                    ]
                    nc.tensor.matmul(
                        ps[:, c * BC : (c + 1) * BC],
                        lhs,
                        rhs,
                        start=(d == 0),
                        stop=(d == 6),
                    )
            nc.scalar.activation(
                ob[:, t * B : (t + 1) * B],
                ps[:],
                mybir.ActivationFunctionType.Sigmoid,
            )
            g0, g1 = ogroups[gidx]
            if t == g1 - 1:
                gidx += 1
                nc.scalar.dma_start(
                    out=y[g0:g1, :, :].rearrange("t p b -> p t b"),
                    in_=ob[:, g0 * NB * BC : g1 * NB * BC].rearrange(
                        "p (t b) -> p t b", t=g1 - g0
                    ),
                )
    nc.compile()
    return nc


def _build_banded(weights: np.ndarray) -> np.ndarray:
    """G[i, d, js, jd] = weight of edge (src row i+d-3, col js) -> (dest row i, col jd).

    dy = 3 - d (dest = src + dy), dx = jd - js, weight index = w[dy+3, dx+3][src_hw].
    """
    w4 = weights.reshape(7, 7, H, W)
    G = np.zeros((H, 7, W, W), np.float32)
    i = np.arange(H)
    for d in range(7):
        r = i + d - 3
        vi = i[(r >= 0) & (r < H)]
        if len(vi) == 0:
            continue
        for dxi in range(7):
            dx = dxi - 3
            js = np.arange(max(0, -dx), W - max(0, dx))
            G[vi[:, None], d, js[None, :], js[None, :] + dx] = w4[6 - d, dxi][
                (vi + d - 3)[:, None], js[None, :]
            ]
    return G


def kernel(x: np.ndarray, weights: np.ndarray) -> np.ndarray:
    global last_exec_time_ns, last_results
    from concourse.bass_utils import run_bass_kernel_spmd

    x = np.ascontiguousarray(x, dtype=np.float32)
    weights = np.ascontiguousarray(weights, dtype=np.float32)

    if MM_MODE not in _cache:
        _cache[MM_MODE] = _build_program(MM_MODE)
    nc = _cache[MM_MODE]

    io_dt = np.float16 if MM_MODE == "f16" else np.float32
    x3 = x.reshape(H, W, B)
    xp = np.zeros((H + 6, W, B), io_dt)
    xp[3 : H + 3] = x3.astype(io_dt)
    G = _build_banded(weights).astype(io_dt)  # [H, 7, W, W] = [i, d, js, jd]

    in_maps = []
    for q in range(NCORES):
        in_maps.append(
            {
                # [col, padded row, batch]
                "xs": np.ascontiguousarray(
                    xp[T * q : T * q + S].transpose(1, 0, 2)
                ),
                # [t, js, d*128 + jd]
                "wb": np.ascontiguousarray(
                    G[T * q : T * q + T].transpose(0, 2, 1, 3).reshape(
                        T, W, 7 * W
                    )
                ),
            }
        )

    trace = os.environ.get("KERNEL_TRACE", "0") == "1"
    res = run_bass_kernel_spmd(
        nc, in_maps, core_ids=list(range(NCORES)), trace=trace
    )
    last_exec_time_ns = res.exec_time_ns
    last_results = res
    out = np.concatenate(
        [
            np.asarray(r["y"]).astype(np.float32).reshape(T * W, B)
            for r in res.results
        ],
        axis=0,
    )
    return out


# revision 30
# speedup vs baseline: 1.0604x; 1.0045x over previous
"""Trainium2 Bass kernel for nn_BCNLayer (locally-connected 7x7 lattice layer + sigmoid).

Math: y[i,j,b] = sigmoid( sum_{dy,dx in [-3,3]} w[dy+3,dx+3][(i-dy)*W + (j-dx)]
                          * x[(i-dy)*W + (j-dx), b] )   (zero outside lattice)

Strategy:
  - 8-way shard over lattice rows (H=128 -> 16 dest rows/core, 22 source rows
    with 3-row halos, zero-padded at the edges).
  - For one dest row i and source-row offset d (7 of them), the contribution is
    a banded 128x128 matrix (band +-3 over lattice columns) applied to the
    source row's [128 cols x B batch] slab:  out[jd, b] += sum_js
    Wband[js, jd] * x[js, b].  That is exactly nc.tensor.matmul(psum, lhsT=Wband,
    rhs=xrow) accumulated over the 7 source rows.
  - Banded matrices are prebuilt on the host (numpy) and DMA'd in; HW executes
    pure DMA + matmul + sigmoid.
  - DMA plan: weights go on the Activation-engine HWDGE ring (fast spin-up, so
    the first LDWEIGHTS fires ~6us earlier than on the SWDGE), x on the SP ring,
    outputs grouped 4 dest-rows per DMA and split across both HWDGE rings.
    Output is bf16 (halves store traffic; sigmoid in bf16 keeps rel err ~2e-3).
"""

import os

import numpy as np

H = 128
W = 128
HW = H * W
B = 1024
NCORES = 8
T = H // NCORES  # dest rows per core = 16
S = T + 6        # source rows per core (halo 3 each side) = 22
BC = 512         # batch chunk (psum bank = 512 fp32)
NB = B // BC     # chunks = 2
OG = 4           # dest rows per output DMA

MM_MODE = os.environ.get("KERNEL_MM_MODE", "f16")

_cache: dict = {}

# filled by the last kernel() call when KERNEL_TRACE=1
last_exec_time_ns = None
last_results = None


def _build_program(mode: str):
    from contextlib import ExitStack

    import concourse.bacc as bacc
    import concourse.mybir as mybir
    import concourse.tile as tile

    nc = bacc.Bacc(
        "TRN2", target_bir_lowering=False, debug=False, num_devices=NCORES
    )
    mm_dt = {
        "f32": mybir.dt.float32,
        "f32r": mybir.dt.float32r,
        "f16": mybir.dt.float16,
    }[mode]
    # xs: [col, padded src row, batch]; wb: [t, src col, d*128 + dest col]
    xs = nc.dram_tensor("xs", [128, S, B], mm_dt, kind="ExternalInput").ap()
    wb = nc.dram_tensor(
        "wb", [T, 128, 7 * 128], mm_dt, kind="ExternalInput"
    ).ap()
    y = nc.dram_tensor(
        "y", [T, 128, B], mybir.dt.bfloat16, kind="ExternalOutput"
    ).ap()

    with tile.TileContext(nc) as tc, ExitStack() as ctx:
        xpool = ctx.enter_context(tc.tile_pool(name="x", bufs=1))
        wpool = ctx.enter_context(tc.tile_pool(name="w", bufs=1))
        mpool = ctx.enter_context(tc.tile_pool(name="m", bufs=1))
        ppool = ctx.enter_context(tc.tile_pool(name="ps", bufs=4, space="PSUM"))
        opool = ctx.enter_context(tc.tile_pool(name="o", bufs=1))

        xt = xpool.tile([128, S * B], mm_dt, tag="xslab")
        wt = wpool.tile([128, T * 7 * 128], mm_dt, tag="wslab")
        ob = opool.tile([128, NB * T * BC], mybir.dt.bfloat16, tag="o")
        xt3 = xt[:].rearrange("p (s b) -> p s b", s=S)

        def _wb(eng, t0, t1):
            eng.dma_start(
                out=wt[:, t0 * 7 * 128 : t1 * 7 * 128].rearrange(
                    "p (t f) -> p t f", t=t1 - t0
                ),
                in_=wb[t0:t1].rearrange("t p f -> p t f"),
            )

        # wb rides the Act ring in need-ordered pieces (wb[t] gates dest-row
        # t's LDWEIGHTS at ~t*3.2us into the stream); x rides the SP ring as
        # full-batch row-range pieces in need order (dest row t consumes
        # padded rows t..t+6 of both chunks). The ramp is supply-bound.
        _wb(nc.scalar, 0, 1)

        # Warm the sigmoid ACT table now — after the wb[0] post (the table
        # load inserts before the first ACTIVATE and must not delay wb[0]),
        # before the rest (it otherwise loads right before the first real
        # sigmoid, stalling the pipeline).
        warm = mpool.tile([128, 1], mybir.dt.float32, tag="warm")
        nc.gpsimd.memset(warm[:], 0.0)
        nc.scalar.activation(warm[:], warm[:], mybir.ActivationFunctionType.Sigmoid)

        for t0, t1 in [(1, 3), (3, 6), (6, 9), (9, 12), (12, 16)]:
            _wb(nc.scalar, t0, t1)
        for lo, hi in [(0, 2), (2, 4), (4, 6), (6, 8), (8, 10), (10, 13),
                       (13, 16), (16, 19), (19, 22)]:
            nc.sync.dma_start(
                out=xt3[:, lo:hi, :],
                in_=xs[:, lo:hi, :],
            )

        # t-outer / batch-chunk-inner: one stationary feeds both 512-col
        # chunks back-to-back (better LDWEIGHTS hiding), and output DMAs
        # cover the full batch. Last groups small for a quick final flush.
        ogroups = [(0, 4), (4, 8), (8, 12), (12, 14), (14, 15), (15, 16)]
        gidx = 0
        for t in range(T):
            ps = ppool.tile([128, NB * BC], mybir.dt.float32, tag="ps")
            for d in range(7):
                lhs = wt[:, (t * 7 + d) * 128 : (t * 7 + d + 1) * 128]
                for c in range(NB):
                    rhs = xt[
                        :, (t + d) * B + c * BC : (t + d) * B + (c + 1) * BC
                    ]
                    nc.tensor.matmul(
                        ps[:, c * BC : (c + 1) * BC],
                        lhs,
                        rhs,
                        start=(d == 0),
                        stop=(d == 6),
                    )
            nc.scalar.activation(
                ob[:, t * B : (t + 1) * B],
                ps[:],
                mybir.ActivationFunctionType.Sigmoid,
            )
            g0, g1 = ogroups[gidx]
            if t == g1 - 1:
                gidx += 1
                # outputs ride the SP ring (idle once x is in) so they never
                # contend with the need-ordered wb pieces on the Act ring
                nc.sync.dma_start(
                    out=y[g0:g1, :, :].rearrange("t p b -> p t b"),
                    in_=ob[:, g0 * NB * BC : g1 * NB * BC].rearrange(
                        "p (t b) -> p t b", t=g1 - g0
                    ),
                )
    nc.compile()
    return nc


def _build_banded(weights: np.ndarray) -> np.ndarray:
    """G[i, d, js, jd] = weight of edge (src row i+d-3, col js) -> (dest row i, col jd).

    dy = 3 - d (dest = src + dy), dx = jd - js, weight index = w[dy+3, dx+3][src_hw].
    """
    w4 = weights.reshape(7, 7, H, W)
    G = np.zeros((H, 7, W, W), np.float32)
    i = np.arange(H)
    for d in range(7):
        r = i + d - 3
        vi = i[(r >= 0) & (r < H)]
        if len(vi) == 0:
            continue
        for dxi in range(7):
            dx = dxi - 3
            js = np.arange(max(0, -dx), W - max(0, dx))
            G[vi[:, None], d, js[None, :], js[None, :] + dx] = w4[6 - d, dxi][
                (vi + d - 3)[:, None], js[None, :]
            ]
    return G


def kernel(x: np.ndarray, weights: np.ndarray) -> np.ndarray:
    global last_exec_time_ns, last_results
    from concourse.bass_utils import run_bass_kernel_spmd

    x = np.ascontiguousarray(x, dtype=np.float32)
    weights = np.ascontiguousarray(weights, dtype=np.float32)

    if MM_MODE not in _cache:
        _cache[MM_MODE] = _build_program(MM_MODE)
    nc = _cache[MM_MODE]

    io_dt = np.float16 if MM_MODE == "f16" else np.float32
    x3 = x.reshape(H, W, B)
    xp = np.zeros((H + 6, W, B), io_dt)
    xp[3 : H + 3] = x3.astype(io_dt)
    G = _build_banded(weights).astype(io_dt)  # [H, 7, W, W] = [i, d, js, jd]

    in_maps = []
    for q in range(NCORES):
        in_maps.append(
            {
                # [col, padded row, batch]
                "xs": np.ascontiguousarray(
                    xp[T * q : T * q + S].transpose(1, 0, 2)
                ),
                # [t, js, d*128 + jd]
                "wb": np.ascontiguousarray(
                    G[T * q : T * q + T].transpose(0, 2, 1, 3).reshape(
                        T, W, 7 * W
                    )
                ),
            }
        )

    trace = os.environ.get("KERNEL_TRACE", "0") == "1"
    res = run_bass_kernel_spmd(
        nc, in_maps, core_ids=list(range(NCORES)), trace=trace
    )
    last_exec_time_ns = res.exec_time_ns
    last_results = res
    out = np.concatenate(
        [
            np.asarray(r["y"]).astype(np.float32).reshape(T * W, B)
            for r in res.results
        ],
        axis=0,
    )
    return out
